# revision 28
# baseline (speedup 1.0000x reference)
"""MultiHeadClassifier (MoE routing) Trainium2 kernel.

Problem: B=65536 samples of dim D=1024, each routed by task_id to one of
T=16 two-layer heads (D->H=128 relu -> C=10). The dense reference computes
all 16 heads for every sample (275 GFLOP); here we route on the host and
compute only each sample's own head (~17 GFLOP), data-parallel with 2 tasks
per NeuronCore across 8 cores.

Per-core budget (trace-measured): the x stream is ~17.3MB bf16 at an
effective 380-450 GB/s (~42-45us) and PE time (160 L1 matmuls + 20 L2) is
~40us -- they are nearly balanced, so the design keeps both saturated:

  - ONE flat DRAM buffer per core in exact consumption order; x arrives as
    ~15 block DMAs (0.25-2MB, tapered sizes) on the sync HWDGE ring: large
    DMAs kill per-DMA SDMA-engine boundary bubbles, the small head blocks
    let the PE start early, the small tail blocks keep the post-stream
    critical path short.
  - ALL weights + biases in ONE DMA at the head of the sync ring (anywhere
    else they drain packet-interleaved with x and land 10+us late). The f32
    biases ride bitcast inside the bf16 buffer, replicated to 512B rows
    (8B-row DMAs are descriptor-RMW-slow).
  - PE warmup matmuls on the weight tile bridge PE-busy from weight arrival
    to steady x delivery, so the HAM clock-gate latches K=8/8 early.
  - Software pipelining: wave w's L2 matmuls + bias adds are emitted after
    wave w+1's L1 matmuls, so the in-order PE queue never stalls on the
    relu ACT producing ht.
  - L2 matmuls go to 32-column strips of the PE array (tile_position) with
    logits accumulating in PSUM partitions 32q..32q+9; the DVE bias-add
    lands them in a [128, OTW] striped SBUF buffer.
  - ONE full-width [128, OTW] out-DMA per slot at the very END: mid-stream
    out-DMAs are poison -- their WAW receipt-wait semaphore descriptors
    block 2-3 SDMA engines for 0.2-2us each, making x-block completion
    sems straggle at 15/16 and stalling the PE (the dominant run-to-run
    variance mode). The full-width dump moves 3.2x the logit bytes but
    uses all 16 SDMA engines with a single receipt stall, draining ~2x
    faster than narrow 10-partition DMAs; the host discards pad rows.

Measured (8 cores, seed-0 data): ~65-68us typical vs 68.9-72.7us baseline;
rel err 3.4e-3 (bf16 matmuls, f32 PSUM accumulation).
"""

import sys

import numpy as np

for _p in ("/opt/trn_rl_repo", "/root/.axon_site/_ro/trn_rl_repo"):
    if _p not in sys.path:
        sys.path.append(_p)

import concourse.bacc as bacc
import concourse.mybir as mybir
from concourse.bass_utils import run_bass_kernel_spmd
from concourse.tile import TileContext

B, D, T, H, C = 65536, 1024, 16, 128, 10
N_CORES = 8
S = T // N_CORES  # task slots per core = 2
DC = D // 128  # d-chunks of 128 = 8
MT = 512  # m-subtile (max fp32 PSUM free dim)
WCOLS = DC * H + 16  # w1 (1024) + w2 (10) + pad

MM_DTYPE = "bf16"

_F32 = mybir.dt.float32
_BF16 = mybir.dt.bfloat16


def _np_bf16():
    import ml_dtypes

    return np.dtype(ml_dtypes.bfloat16)


def _plan_blocks(M):
    """Tapered block sizes summing to M: small head (PE starts early),
    1024-col middle, descending tail (short post-stream critical path)."""
    assert M % 32 == 0
    if M <= 1024:
        return [M]
    head, tail = [512, 512], [512, 256, 128]
    rem = M - sum(head) - sum(tail)
    if rem < 0:
        head, tail = [], [512, 256, 128]
        rem = M - sum(tail)
        if rem < 0:
            return [M - 256, 256] if M > 512 else [M]
    mids = []
    while rem > 0:
        c = min(1024, rem)
        if rem - c and rem - c < 256:
            c = rem - 256
        mids.append(c)
        rem -= c
    return head + mids + tail


def _chunks(total, step):
    out = []
    p = 0
    while p < total:
        c = min(step, total - p)
        out.append((p, c))
        p += c
    return out


def _build(M_task, mm_dtype=MM_DTYPE):
    assert mm_dtype == "bf16"
    blocks = _plan_blocks(M_task)
    # interleaved issue order: (b0,s0), (b0,s1), (b1,s0), ...
    sched = [(bi, s) for bi in range(len(blocks)) for s in range(S)]
    # flat x layout: per (block, slot) region [128, DC*xl], partition-major
    offs = {}
    off = 0
    starts = np.concatenate([[0], np.cumsum(blocks)]).astype(int)
    for bi, s in sched:
        xl = blocks[bi]
        offs[(bi, s)] = off
        off += 128 * DC * xl
    total_x = off

    n_sub = sum(-(-b // MT) for b in blocks)
    OTW = -(-n_sub // 4) * MT

    nc = bacc.Bacc(None, target_bir_lowering=False)
    xL = nc.declare_dram_parameter("xL", [total_x], _BF16, isOutput=False)
    # wall = [wcat0 | wcat1 | biases(f32 bitcast)] in one [128, row] DMA
    wall = nc.declare_dram_parameter(
        "wall", [128, 2 * WCOLS + 4 * 128], _BF16, isOutput=False
    )
    outT = nc.declare_dram_parameter("outT", [S, 128, OTW], _F32, isOutput=True)

    relu = mybir.ActivationFunctionType.Relu
    N_WARMUP = 8

    with TileContext(nc) as tc:
        with (
            tc.tile_pool(name="wpool", bufs=1) as wpool,
            tc.tile_pool(name="xpool", bufs=1) as xpool,
            tc.tile_pool(name="hpool", bufs=8) as hpool,
            tc.tile_pool(name="opool", bufs=1) as opool,
            tc.tile_pool(name="psum1", bufs=4, space="PSUM") as psum1,
            tc.tile_pool(name="psum2", bufs=4, space="PSUM") as psum2,
        ):
            # ALL weights + biases in ONE DMA, FIRST on the sync ring,
            # ahead of the x flood (anywhere else they drain packet-
            # interleaved with x and land 10+us late, stalling compute).
            # Biases ride as f32 bitcast into the bf16 wall, replicated to
            # 512B rows (8B-row DMAs are descriptor-RMW-slow)
            wallt = wpool.tile([128, 2 * WCOLS + 4 * 128], _BF16, tag="wall")
            nc.sync.dma_start(wallt, wall[:, :])
            bt = wallt[:, 2 * WCOLS :].bitcast(_F32)  # [128, 256] f32
            wts = [
                (
                    wallt[:, :WCOLS],
                    bt[:, 0:1],
                    [bt[32 * q : 32 * q + C, 64:65] for q in range(4)],
                ),
                (
                    wallt[:, WCOLS : 2 * WCOLS],
                    bt[:, 128:129],
                    [bt[32 * q : 32 * q + C, 192:193] for q in range(4)],
                ),
            ]

            # all x block DMAs up-front on the sync HWDGE ring, in
            # consumption order; blocks > 512 cols ship as TWO half-DMAs so
            # the completion-receipt latency (~0.2-2us under HBM load) of
            # the half PE waits on has already settled when it gets there
            xts = {}
            for bi, s in sched:
                xl = blocks[bi]
                o = offs[(bi, s)]
                halves = []
                n_half = 1
                hl = xl // n_half
                for h in range(n_half):
                    xt = xpool.tile(
                        [128, DC * hl], _BF16,
                        tag=f"x{bi}_{s}_{h}", name=f"x{bi}_{s}_{h}",
                    )
                    oh = o + h * 128 * DC * hl
                    nc.sync.dma_start(
                        xt,
                        xL[oh : oh + 128 * DC * hl].rearrange("(p k) -> p k", p=128),
                    )
                    halves.append(xt)
                xts[(bi, s)] = (halves, hl)

            # PE warmup on slot-0 weights (land ~9us; x flows from ~11us):
            # garbage matmuls into a scratch bank release the HAM clock gate
            # and bridge PE-busy until delivery is continuous
            wps = psum2.tile([128, MT], _F32, tag="ps2", name="warm")
            w0 = wts[0][0]
            for _ in range(N_WARMUP):
                nc.tensor.matmul(wps[:], w0[:, :128], w0[:, :MT], start=True, stop=True)

            # Software pipeline: wave w's L2 matmuls/bias/out are emitted
            # after wave w+1's L1 matmuls, so the in-order PE queue never
            # stalls on the relu ACT that produces ht.
            pending = None

            def flush_l2(nxt):
                nonlocal pending
                if pending is None:
                    pending = nxt
                    return
                pwt, phts, pps2, pwave, pot, pb2s, pouts = pending
                for j, (m0, mt, q, c0) in enumerate(pwave):
                    # L2 into 32-col strip q of the PE array, PSUM partitions
                    # 32q..32q+C: the wave's L2 matmuls use disjoint subarray
                    # columns and run concurrently; the strip layout also
                    # spreads the final out-DMA across 40 SBUF partitions
                    nc.tensor.matmul(
                        pps2[j][32 * q : 32 * q + C, :mt],
                        pwt[:, DC * H : DC * H + C],
                        phts[j][:, :mt],
                        start=True,
                        stop=True,
                        tile_position=(0, 32 * q),
                    )
                for j, (m0, mt, q, c0) in enumerate(pwave):
                    nc.vector.tensor_tensor(
                        pot[32 * q : 32 * q + C, c0 : c0 + mt],
                        pps2[j][32 * q : 32 * q + C, :mt],
                        pb2s[q].to_broadcast([C, mt]),
                        mybir.AluOpType.add,
                    )
                for dma_args in pouts:
                    # sync ring: out descriptors queue behind the x stream
                    # and drain right after it; ot tiles are never recycled
                    # (distinct tags) so compute never waits on these
                    nc.sync.dma_start(*dma_args)
                pending = nxt

            WAVE = 2
            ots = [
                opool.tile([128, OTW], _F32, tag=f"o{s}", name=f"o{s}")
                for s in range(S)
            ]
            gctr = [0] * S
            for bi, s in sched:
                xl = blocks[bi]
                x0 = starts[bi]
                wt, b1t, b2s = wts[s]
                halves, hl = xts[(bi, s)]
                ot = ots[s]
                subs = _chunks(xl, MT)
                for w0i in range(0, len(subs), WAVE):
                    wave = []
                    for m0, mt in subs[w0i : w0i + WAVE]:
                        g = gctr[s]
                        gctr[s] += 1
                        wave.append((m0, mt, g % 4, (g // 4) * MT))
                    ps1s = [
                        psum1.tile([H, MT], _F32, tag="ps1", name=f"ps1_{j}")
                        for j in range(len(wave))
                    ]
                    for dc in range(DC):
                        lhs = wt[:, dc * H : (dc + 1) * H]
                        for j, (m0, mt, q, c0) in enumerate(wave):
                            hi = m0 // hl
                            ml = m0 - hi * hl
                            nc.tensor.matmul(
                                ps1s[j][:, :mt],
                                lhs,
                                halves[hi][:, dc * hl + ml : dc * hl + ml + mt],
                                start=(dc == 0),
                                stop=(dc == DC - 1),
                            )
                    # ONE out-DMA per slot, after its last block: out-DMAs
                    # write HBM from ~10 partitions (2-3 SDMA engines) and
                    # their WAW receipt-wait sem descriptor blocks those
                    # engines' queues 0.2-2us; mid-stream that makes the x
                    # block sems straggle at 15/16 and stalls the PE
                    last_wave = w0i + WAVE >= len(subs)
                    # one full-width [128, OTW] dump per slot: 3.2x the
                    # bytes of the logits but a single DMA across all 16
                    # SDMA engines with ONE receipt stall -- drains ~2x
                    # faster than 4 narrow strip-DMAs on 3 engines
                    outs = (
                        [(outT[s], ot[:])]
                        if (last_wave and bi == len(blocks) - 1)
                        else []
                    )
                    hts = []
                    for j, (m0, mt, q, c0) in enumerate(wave):
                        ht = hpool.tile([H, MT], _BF16, tag="h")
                        nc.scalar.activation(
                            ht[:, :mt], ps1s[j][:, :mt], relu, bias=b1t
                        )
                        hts.append(ht)
                    ps2s = [
                        psum2.tile([128, MT], _F32, tag="ps2", name=f"ps2_{j}")
                        for j in range(len(wave))
                    ]
                    flush_l2((wt, hts, ps2s, wave, ot, b2s, outs))
            flush_l2(None)
    nc.compile()
    return nc


def _prepare(x, task_id, W1, b1, W2, b2, mm_dtype=MM_DTYPE):
    """Host-side routing: returns (in_maps, idx, counts, M_task)."""
    bf16 = _np_bf16()
    x = np.ascontiguousarray(np.asarray(x, dtype=np.float32))
    task_id = np.asarray(task_id).astype(np.int64)
    W1 = np.asarray(W1, dtype=np.float32)
    b1 = np.asarray(b1, dtype=np.float32)
    W2 = np.asarray(W2, dtype=np.float32)
    b2 = np.asarray(b2, dtype=np.float32)

    order = np.argsort(task_id, kind="stable")
    counts = np.bincount(task_id, minlength=T)
    starts_t = np.concatenate([[0], np.cumsum(counts)])
    M_task = max(128, int(-(-int(counts.max()) // 32) * 32))

    blocks = _plan_blocks(M_task)
    sched = [(bi, s) for bi in range(len(blocks)) for s in range(S)]
    bstarts = np.concatenate([[0], np.cumsum(blocks)]).astype(int)

    # idx[t] = sample rows for task t, padded with row 0 (discarded later)
    idx = np.zeros((T, M_task), dtype=np.int64)
    for t in range(T):
        idx[t, : counts[t]] = order[starts_t[t] : starts_t[t + 1]]

    in_maps = []
    for c in range(N_CORES):
        ts_c = [S * c + s for s in range(S)]
        # xT[s] = [DC, 128, M] (d-major within chunk on axis 1)
        xTs = []
        for s in range(S):
            xg = x[idx[ts_c[s]]].astype(bf16)  # [M, D]
            xTs.append(np.ascontiguousarray(xg.T).reshape(DC, 128, M_task))
        xL = np.empty(sum(128 * DC * b for b in blocks) * S, dtype=bf16)
        off = 0
        for bi, s in sched:
            xl = blocks[bi]
            x0 = bstarts[bi]
            n_half = 1
            hl = xl // n_half
            for h in range(n_half):
                # region [128, DC, hl] partition-major
                reg = xTs[s][:, :, x0 + h * hl : x0 + (h + 1) * hl].transpose(
                    1, 0, 2
                )
                n = 128 * DC * hl
                xL[off : off + n] = reg.reshape(-1)
                off += n

        wall = np.zeros((128, 2 * WCOLS + 4 * 128), dtype=bf16)
        bcat = np.zeros((128, S * 128), dtype=np.float32)
        for s in range(S):
            t = ts_c[s]
            # w1 [D,H] -> [128, DC*H] partition-major
            wall[:, s * WCOLS : s * WCOLS + DC * H] = (
                W1[t].reshape(DC, 128, H).transpose(1, 0, 2).reshape(128, DC * H)
            ).astype(bf16)
            wall[:, s * WCOLS + DC * H : s * WCOLS + DC * H + C] = W2[t].astype(bf16)
            bcat[:, s * 128 : s * 128 + 64] = b1[t][:, None]
            for q in range(4):
                bcat[32 * q : 32 * q + C, s * 128 + 64 : s * 128 + 128] = b2[
                    t
                ][:, None]
        wall[:, 2 * WCOLS :] = bcat.view(bf16).reshape(128, 4 * 128)

        in_maps.append({"xL": xL, "wall": wall})
    return in_maps, idx, counts, M_task


def _unshard(results, idx, counts, b_total=B):
    M_task = idx.shape[1]
    blocks = _plan_blocks(M_task)
    bstarts = np.concatenate([[0], np.cumsum(blocks)]).astype(int)
    # subtile g of a slot lives at outT[s, g % 4, :, (g // 4) * MT :]
    smap = []
    g = 0
    for bi, xl in enumerate(blocks):
        for m0, mt in _chunks(xl, MT):
            smap.append((bstarts[bi] + m0, mt, g % 4, (g // 4) * MT))
            g += 1
    out = np.empty((b_total, C), dtype=np.float32)
    for c in range(N_CORES):
        yT = np.asarray(results[c]["outT"])  # [S, 128, OTW]
        for s in range(S):
            y = np.empty((M_task, C), dtype=np.float32)
            for m0, mt, q, c0 in smap:
                y[m0 : m0 + mt] = yT[s, 32 * q : 32 * q + C, c0 : c0 + mt].T
            t = S * c + s
            cnt = counts[t]
            out[idx[t, :cnt]] = y[:cnt]
    return out


def kernel(x, task_id, W1, b1, W2, b2):
    in_maps, idx, counts, M_task = _prepare(x, task_id, W1, b1, W2, b2)
    nc = _build(M_task)
    try:
        res = run_bass_kernel_spmd(nc, in_maps, list(range(N_CORES)))
    except Exception:
        # transient NRT device hiccups (e.g. NRT_EXEC_UNIT_UNRECOVERABLE)
        # have been observed to succeed on retry
        res = run_bass_kernel_spmd(nc, in_maps, list(range(N_CORES)))
    return _unshard(res.results, idx, counts, b_total=np.asarray(task_id).shape[0])
